# revision 16
# baseline (speedup 1.0000x reference)
"""Mamba block Trainium2 kernel, 8-way tensor-parallel over d_inner.

Shapes (hardcoded from the problem spec):
  hidden_states [2, 1024, 1024], d_model=1024, d_inner=2048, d_state=16,
  dt_rank=64, d_conv=4.  Each core owns DL=256 d_inner channels (two
  128-partition halves, "dt0"/"dt1", processed merged along the free dim).

Engine assignment (per-core):
  PE   : in_proj / x_proj / dt_proj / out_proj matmuls + y-accumulate over n
         (identity-stationary matmuls into PSUM)
  ACT  : conv taps (scaled copies), Silu, softplus (Exp+Ln), per-n
         exp(A*delta), h fp32->fp16 casts, PSUM->SBUF staging copies
  DVE  : conv tap adds, du/dbu/ch fp16 muls, the sequential scans, gating
  POOL : collectives + chain-breaker memsets only (DVE+POOL share SBUF
         ports; concurrent streaming on both just splits the bandwidth)

All elementwise traffic is uniform fp16 (measured: [128,1024] TT mul 683ns
vs ~2300-3400ns for mixed-dtype variants; scan fp16->fp32 2258ns vs 3942ns
with a bf16 operand).  The scan keeps an fp32 internal state/output.
The scan hot loop runs on [128, 2048] tiles covering both channel halves;
a zeroed dA column at position L breaks the recurrence between them.

Pipelining: head(b0) -> AR1(b0) overlapped with head(b1); delta(b1) emitted
inside scan(b0); gate/out_proj/ReduceScatter of b0 emitted inside scan(b1).
"""
import sys, os
sys.path.insert(0, "/opt/trn_rl_repo")
import numpy as np

import concourse.bass as bass
import concourse.bacc as bacc
import concourse.mybir as mybir
import concourse.tile as tile

F32 = mybir.dt.float32
F16 = mybir.dt.float16
AF = mybir.ActivationFunctionType
OP = mybir.AluOpType

B, L, D, DI, NST, RNK, KC = 2, 1024, 1024, 2048, 16, 64, 4
NC_ = 8
DL = DI // NC_          # 256 local channels
T = B * L               # 2048 tokens
PADL = L + KC - 1       # 1027
L2 = 2 * L              # merged dt0|dt1 free extent


def build_nc():
    nc = bacc.Bacc()
    dp = nc.declare_dram_parameter
    hsT = dp("hsT", [D, T], F16, isOutput=False)             # hidden^T fp16
    wxz = dp("wxzT", [8, 128, 512], F16, isOutput=False)     # in_proj^T k-tiles
    xpw = dp("xpwT", [2, 128, 96], F16, isOutput=False)      # x_proj^T k-tiles
    dtw = dp("dtwT", [RNK, DL], F16, isOutput=False)         # dt_proj^T
    wo = dp("woT", [2, 128, D], F16, isOutput=False)         # out_proj^T k-tiles
    cw = dp("convw", [2, 128, KC], F32, isOutput=False)
    cb = dp("convb", [2, 128, 1], F32, isOutput=False)
    db2 = dp("dtb2p", [2, 128, 1], F32, isOutput=False)      # +2*dt_proj_b
    av = dp("Aneg", [2, 128, NST], F32, isOutput=False)      # -exp(A_log)
    dv = dp("Dvec", [2, 128, 1], F32, isOutput=False)
    idm = dp("ident", [128, 128], F32, isOutput=False)
    outp = dp("out", [B, 128, D], F16, isOutput=True)        # per-core RS slice

    ar1_in = [nc.dram_tensor(f"ar1_in{b}", [96, L], F16) for b in range(B)]
    ar1_out = [nc.dram_tensor(f"ar1_out{b}", [96, L], F16, addr_space="Shared")
               for b in range(B)]
    ar2_in = [nc.dram_tensor(f"ar2_in{b}", [L, D], F16) for b in range(B)]
    ar2_out = [nc.dram_tensor(f"ar2_out{b}", [128, D], F16) for b in range(B)]

    with tile.TileContext(nc) as tc:
        with tc.tile_pool(name="wp", bufs=1) as wp, \
             tc.tile_pool(name="data", bufs=1) as dpool, \
             tc.tile_pool(name="stream", bufs=4) as stream, \
             tc.tile_pool(name="scan", bufs=2) as scp, \
             tc.tile_pool(name="stage", bufs=2) as stg, \
             tc.tile_pool(name="psA", bufs=4, space="PSUM") as psA, \
             tc.tile_pool(name="psy", bufs=2, space="PSUM") as psy:

            # ---- weights / constants ----
            # wxz first (in_proj needs it immediately); the rest are deferred
            # until after the first in_proj k-sweep is emitted.
            wxz_sb = wp.tile([128, 8 * 512], F16)
            for k in range(8):
                nc.sync.dma_start(wxz_sb[:, 512 * k:512 * (k + 1)], wxz[k])
            xpw_sb = wp.tile([128, 2 * 96], F16)
            dtw_sb = wp.tile([RNK, DL], F16)
            wo_sb = wp.tile([128, 2 * D], F16)
            cw_sb = wp.tile([128, 2 * KC], F32)
            cb_sb = wp.tile([128, 2], F32)
            db2_sb = wp.tile([128, 2], F32)
            av_sb = wp.tile([128, 2 * NST], F32)
            dv_sb = wp.tile([128, 2], F32)
            id_sb = wp.tile([128, 128], F32)
            id16 = wp.tile([128, 128], F16)

            def emit_weight_tail():
                nc.sync.dma_start(dtw_sb[:], dtw[:])
                for k in range(2):
                    nc.sync.dma_start(xpw_sb[:, 96 * k:96 * (k + 1)], xpw[k])
                    nc.sync.dma_start(wo_sb[:, D * k:D * (k + 1)], wo[k])
                    nc.sync.dma_start(cw_sb[:, KC * k:KC * (k + 1)], cw[k])
                    nc.sync.dma_start(cb_sb[:, k:k + 1], cb[k])
                    nc.sync.dma_start(db2_sb[:, k:k + 1], db2[k])
                    nc.sync.dma_start(av_sb[:, NST * k:NST * (k + 1)], av[k])
                    nc.sync.dma_start(dv_sb[:, k:k + 1], dv[k])
                nc.sync.dma_start(id_sb[:], idm[:])
                nc.vector.tensor_copy(id16[:], id_sb[:])

            cwv = cw_sb.rearrange("p (k m) -> p k m", k=2)
            wxzv = wxz_sb.rearrange("p (k m) -> p k m", k=8)
            wov = wo_sb.rearrange("p (k m) -> p k m", k=2)

            # ---- persistent per-batch activations (dt-merged free dim) ----
            xpad = [[dpool.tile([128, PADL], F16, name=f"xpad{b}_{i}")
                     for i in range(2)] for b in range(B)]
            sg16 = [dpool.tile([128, L2], F16, name=f"sg{b}") for b in range(B)]
            u16 = [dpool.tile([128, L2], F16, name=f"u{b}") for b in range(B)]
            delta = [dpool.tile([128, L2], F32, name=f"delta{b}")
                     for b in range(B)]
            du16 = [dpool.tile([128, L2], F16, name=f"du{b}") for b in range(B)]
            yg16 = [dpool.tile([128, L2], F16, name=f"yg{b}") for b in range(B)]
            xdbl = [dpool.tile([RNK, L], F16, name=f"xdbl{b}") for b in range(B)]

            def emit_head(b):
                for dt_ in range(2):
                    nc.vector.memset(xpad[b][dt_][:, 0:KC - 1], 0.0)
                # in_proj in half-token sweeps; conv/silu/x_proj follow each
                # sweep so x_proj(b) is ready as soon as in_proj(b) drains.
                for th in range(2):
                    hts = slice(L * b + 512 * th, L * b + 512 * (th + 1))
                    ps4 = [psA.tile([128, 512], F32, name=f"psip{i}", tag="psA")
                           for i in range(4)]   # x0 x1 z0 z1
                    for k in range(8):
                        hk = stream.tile([128, 512], F16, name="hk", tag="hst")
                        nc.sync.dma_start(hk[:], hsT[128 * k:128 * (k + 1), hts])
                        for g in range(4):
                            nc.tensor.matmul(
                                ps4[g][:], wxzv[:, k, 128 * g:128 * (g + 1)],
                                hk[:], start=(k == 0), stop=(k == 7))
                    if b == 0 and th == 0:
                        emit_weight_tail()
                    o = KC - 1 + 512 * th
                    nc.scalar.copy(xpad[b][0][:, o:o + 512], ps4[0][:])
                    nc.scalar.copy(xpad[b][1][:, o:o + 512], ps4[1][:])
                    nc.scalar.activation(sg16[b][:, 512 * th:512 * (th + 1)],
                                         ps4[2][:], AF.Silu)
                    nc.scalar.activation(
                        sg16[b][:, L + 512 * th:L + 512 * (th + 1)],
                        ps4[3][:], AF.Silu)
                    # conv + silu for this half-token range
                    c0 = 512 * th
                    for dt_ in range(2):
                        taps = [stg.tile([128, 512], F16, name=f"tap{dt_}_{k}",
                                         tag=f"tap{dt_}_{k}", bufs=2)
                                for k in range(KC)]
                        for k in range(KC):
                            nc.scalar.activation(
                                taps[k][:], xpad[b][dt_][:, c0 + k:c0 + k + 512],
                                AF.Identity, scale=cwv[:, dt_, k:k + 1])
                        t01 = stg.tile([128, 512], F16, name="t01",
                                       tag=f"t01_{dt_}", bufs=2)
                        t23 = stg.tile([128, 512], F16, name="t23",
                                       tag=f"t23_{dt_}", bufs=2)
                        nc.vector.tensor_add(t01[:], taps[0][:], taps[1][:])
                        nc.vector.tensor_add(t23[:], taps[2][:], taps[3][:])
                        acc = stg.tile([128, 512], F16, name="acc",
                                       tag=f"acc{dt_}", bufs=2)
                        nc.vector.tensor_add(acc[:], t01[:], t23[:])
                        nc.scalar.activation(
                            u16[b][:, L * dt_ + c0:L * dt_ + c0 + 512],
                            acc[:], AF.Silu, bias=cb_sb[:, dt_:dt_ + 1])
                # x_proj after both sweeps so its matmuls never block the
                # PE queue while waiting on the conv chain
                for th in range(2):
                    c0 = 512 * th
                    ps96 = psA.tile([96, 512], F32, name="ps96", tag="psA")
                    for k in range(2):
                        nc.tensor.matmul(
                            ps96[:], xpw_sb[:, 96 * k:96 * (k + 1)],
                            u16[b][:, L * k + c0:L * k + c0 + 512],
                            start=(k == 0), stop=(k == 1))
                    st = stg.tile([96, 512], F16, name="st_xp", tag="xp")
                    nc.scalar.copy(st[:], ps96[:])
                    nc.sync.dma_start(
                        ar1_in[b][:, 512 * th:512 * (th + 1)], st[:])
                if os.environ.get("MAMBA_NO_AR"):
                    nc.sync.dma_start(ar1_out[b][:], ar1_in[b][:])
                else:
                    nc.gpsimd.collective_compute(
                        "AllReduce", OP.add,
                        replica_groups=[list(range(NC_))],
                        ins=[ar1_in[b][:]], outs=[ar1_out[b][:]])

            def emit_delta(b):
                nc.sync.dma_start(xdbl[b][:], ar1_out[b][0:RNK, :])
                for dt_ in range(2):
                    dsl = delta[b][:, L * dt_:L * (dt_ + 1)]
                    psd = psA.tile([128, 512], F32, name="psda", tag="psA")
                    psdb = psA.tile([128, 512], F32, name="psdb", tag="psA")
                    nc.tensor.matmul(psd[:], dtw_sb[:, 128 * dt_:128 * (dt_ + 1)],
                                     xdbl[b][:, 0:512], start=True, stop=True)
                    nc.tensor.matmul(psdb[:], dtw_sb[:, 128 * dt_:128 * (dt_ + 1)],
                                     xdbl[b][:, 512:L], start=True, stop=True)
                    # delta = softplus(dt_raw + 2*dt_proj_b) = ln(exp(..)+1)
                    nc.scalar.activation(dsl[:, 0:512], psd[:], AF.Exp,
                                         bias=db2_sb[:, dt_:dt_ + 1])
                    nc.scalar.activation(dsl[:, 512:L], psdb[:], AF.Exp,
                                         bias=db2_sb[:, dt_:dt_ + 1])
                # Ln ops grouped after both Exp blocks (one table switch)
                nc.scalar.activation(delta[b][:], delta[b][:], AF.Ln, bias=1.0)
                # du = delta * u   (fp32 x fp16 -> fp16, merged)
                nc.vector.tensor_mul(du16[b][:], delta[b][:], u16[b][:])

            def emit_gate(b):
                # y = (u*D + y_scan) * silu(z)  -- frees py[b] PSUM banks
                for dt_ in range(2):
                    lsl = slice(L * dt_, L * (dt_ + 1))
                    t16 = stg.tile([128, L], F16, name="t16", tag="t16")
                    nc.vector.scalar_tensor_tensor(
                        t16[:], u16[b][:, lsl], dv_sb[:, dt_:dt_ + 1],
                        py[b][dt_][:], op0=OP.mult, op1=OP.add)
                    nc.vector.tensor_mul(yg16[b][:, lsl], t16[:],
                                         sg16[b][:, lsl])

            def emit_out_tile(b, tt):
                pso = [psA.tile([128, 512], F32, name=f"pso{i}", tag="psA")
                       for i in range(2)]
                for k in range(2):
                    ysl = yg16[b][:, L * k + 128 * tt:L * k + 128 * (tt + 1)]
                    for hh in range(2):
                        nc.tensor.matmul(
                            pso[hh][:], ysl,
                            wov[:, k, 512 * hh:512 * (hh + 1)],
                            start=(k == 0), stop=(k == 1))
                st = stg.tile([128, D], F16, name="st_op", tag="op")
                nc.scalar.copy(st[:, 0:512], pso[0][:])
                nc.scalar.copy(st[:, 512:D], pso[1][:])
                nc.sync.dma_start(ar2_in[b][128 * tt:128 * (tt + 1), :], st[:])

            def emit_rs(b):
                if os.environ.get("MAMBA_NO_AR"):
                    nc.sync.dma_start(ar2_out[b][:], ar2_in[b][0:128, :])
                else:
                    nc.gpsimd.collective_compute(
                        "ReduceScatter", OP.add,
                        replica_groups=[list(range(NC_))],
                        ins=[ar2_in[b][:]], outs=[ar2_out[b][:]])
                nc.sync.dma_start(outp[b], ar2_out[b][:])

            py = [None, None]
            # per-n scan-state carries for the b1 half-token passes
            carry = [wp.tile([128, NST], F32, name="carry0"),
                     wp.tile([128, NST], F32, name="carry1")]

            def emit_scan_pass(b, pass_, width, insert=None):
                """One n-loop over token columns [c0, c0+width) of batch b,
                dt-merged: op tiles are [128, 2*width] = dt0|dt1.
                pass_ 0 with width=L is the full-batch single pass.
                For width=512: pass 0 saves end-of-pass state into carry[];
                pass 1 seeds from it (initial= for dt0, dbu fixup for dt1)."""
                c0 = 512 * pass_
                W2 = 2 * width
                for n in range(NST):
                    bct = scp.tile([128, W2], F16, name="bct", tag="bct",
                                   bufs=3)
                    bv = bct.rearrange("p (two l) -> p two l", two=2)
                    nc.sync.dma_start(
                        bv[:, 0], ar1_out[b][RNK + n:RNK + n + 1, c0:c0 + width]
                        .broadcast_to((128, width)))
                    nc.sync.dma_start(
                        bv[:, 1],
                        ar1_out[b][RNK + NST + n:RNK + NST + n + 1, c0:c0 + width]
                        .broadcast_to((128, width)))
                    brep = bct[:, 0:width].unsqueeze(1).broadcast_to(
                        (128, 2, width))
                    crep = bct[:, width:W2].unsqueeze(1).broadcast_to(
                        (128, 2, width))
                    dA16 = scp.tile([128, W2], F16, name="dA", tag="dA", bufs=3)
                    nc.scalar.activation(
                        dA16[:, 0:width], delta[b][:, c0:c0 + width], AF.Exp,
                        scale=av_sb[:, n:n + 1])
                    # dt1 exp: skip col `width` only when its dA must be zero
                    # with no carry (start of the dt1 sequence)
                    d1lo = width + (0 if pass_ == 1 else 1)
                    nc.scalar.activation(
                        dA16[:, d1lo:W2],
                        delta[b][:, L + c0 + (d1lo - width):L + c0 + width],
                        AF.Exp, scale=av_sb[:, NST + n:NST + n + 1])
                    dbu = scp.tile([128, W2], F16, name="dbu", tag="dbu",
                                   bufs=3)
                    du_v = du16[b].rearrange("p (two l) -> p two l", two=2)
                    dbu_v = dbu.rearrange("p (two l) -> p two l", two=2)
                    nc.vector.tensor_mul(dbu_v[:], du_v[:, :, c0:c0 + width],
                                         brep)
                    if pass_ == 1:
                        # seed dt1 from carried state, then break the chain
                        nc.vector.scalar_tensor_tensor(
                            dbu[:, width:width + 1], dA16[:, width:width + 1],
                            carry[1][:, n:n + 1], dbu[:, width:width + 1],
                            op0=OP.mult, op1=OP.add)
                    nc.vector.memset(dA16[:, width:width + 1], 0.0)
                    h = scp.tile([128, W2], F32, name="h", tag="h", bufs=2)
                    init = carry[0][:, n:n + 1] if pass_ == 1 else 0.0
                    nc.vector.tensor_tensor_scan(
                        h[:], dA16[:], dbu[:], init, op0=OP.mult, op1=OP.add)
                    if width != L and pass_ == 0:
                        nc.scalar.copy(carry[0][:, n:n + 1],
                                       h[:, width - 1:width])
                        nc.scalar.copy(carry[1][:, n:n + 1],
                                       h[:, W2 - 1:W2])
                    h16 = scp.tile([128, W2], F16, name="h16", tag="h16",
                                   bufs=2)
                    nc.scalar.copy(h16[:], h[:])
                    ch = scp.tile([128, W2], F16, name="ch", tag="ch", bufs=2)
                    h16_v = h16.rearrange("p (two l) -> p two l", two=2)
                    ch_v = ch.rearrange("p (two l) -> p two l", two=2)
                    nc.vector.tensor_mul(ch_v[:], h16_v[:], crep)
                    for dt_ in range(2):
                        for q0 in range(0, width, 512):
                            nc.tensor.matmul(
                                py[b][dt_][:, c0 + q0:c0 + q0 + 512], id16[:],
                                ch[:, width * dt_ + q0:width * dt_ + q0 + 512],
                                start=(n == 0), stop=(n == NST - 1))
                    if insert:
                        insert(n)

            def emit_gate_half(b, pass_, width):
                c0 = 512 * pass_
                for dt_ in range(2):
                    t16 = stg.tile([128, width], F16, name="t16", tag="t16")
                    nc.vector.scalar_tensor_tensor(
                        t16[:], u16[b][:, L * dt_ + c0:L * dt_ + c0 + width],
                        dv_sb[:, dt_:dt_ + 1], py[b][dt_][:, c0:c0 + width],
                        op0=OP.mult, op1=OP.add)
                    nc.vector.tensor_mul(
                        yg16[b][:, L * dt_ + c0:L * dt_ + c0 + width],
                        t16[:], sg16[b][:, L * dt_ + c0:L * dt_ + c0 + width])

            emit_head(0)
            emit_head(1)
            emit_delta(0)
            py[0] = [psy.tile([128, L], F32, name=f"py0_{i}", tag="psy")
                     for i in range(2)]

            def ins_b0(n):
                if n == 2:
                    emit_delta(1)
            emit_scan_pass(0, 0, L, insert=ins_b0)
            emit_gate_half(0, 0, L)
            py[1] = [psy.tile([128, L], F32, name=f"py1_{i}", tag="psy")
                     for i in range(2)]

            def ins_b1a(n):
                # b0's out_proj + both half-RS, all hidden under this pass
                if 2 <= n <= 9:
                    emit_out_tile(0, n - 2)
                if n == 11:
                    emit_rs(0)
            emit_scan_pass(1, 0, 512, insert=ins_b1a)

            def ins_b1b(n):
                if n == 0:
                    emit_gate_half(1, 0, 512)
                if 1 <= n <= 4:
                    emit_out_tile(1, n - 1)
            emit_scan_pass(1, 1, 512, insert=ins_b1b)
            emit_gate_half(1, 1, 512)
            for tt in range(4, 8):
                emit_out_tile(1, tt)
            emit_rs(1)
    nc.finalize()
    return nc


def make_in_maps(inputs):
    hs = np.asarray(inputs["hidden_states"], np.float32)
    ipw = np.asarray(inputs["in_proj_w"], np.float32)
    cw = np.asarray(inputs["conv_w"], np.float32)
    cb = np.asarray(inputs["conv_b"], np.float32)
    xpw = np.asarray(inputs["x_proj_w"], np.float32)
    dtw = np.asarray(inputs["dt_proj_w"], np.float32)
    dtb = np.asarray(inputs["dt_proj_b"], np.float32)
    alog = np.asarray(inputs["A_log"], np.float32)
    dvec = np.asarray(inputs["D"], np.float32)
    wo = np.asarray(inputs["out_proj_w"], np.float32)

    hsT = np.ascontiguousarray(hs.transpose(2, 0, 1).reshape(D, T)).astype(np.float16)
    ident = np.eye(128, dtype=np.float32)

    in_maps = []
    for c in range(NC_):
        sl = slice(DL * c, DL * (c + 1))
        wxzT = np.concatenate([ipw[sl].T, ipw[DI + DL * c: DI + DL * (c + 1)].T],
                              axis=1)                      # [1024, 512]
        m = {
            "hsT": hsT,
            "wxzT": np.ascontiguousarray(wxzT.reshape(8, 128, 512)).astype(np.float16),
            "xpwT": np.ascontiguousarray(xpw[:, sl].T.reshape(2, 128, 96)).astype(np.float16),
            "dtwT": np.ascontiguousarray(dtw[sl].T).astype(np.float16),
            "woT": np.ascontiguousarray(wo[:, sl].T.reshape(2, 128, D)).astype(np.float16),
            "convw": np.ascontiguousarray(cw[sl, 0, :].reshape(2, 128, KC)),
            "convb": np.ascontiguousarray(cb[sl].reshape(2, 128, 1)),
            "dtb2p": np.ascontiguousarray((2.0 * dtb[sl]).reshape(2, 128, 1)),
            "Aneg": np.ascontiguousarray((-np.exp(alog[sl])).reshape(2, 128, NST)),
            "Dvec": np.ascontiguousarray(dvec[sl].reshape(2, 128, 1)),
            "ident": ident,
        }
        in_maps.append(m)
    return in_maps


def assemble_output(results):
    out = np.zeros((B, L, D), np.float32)
    for c in range(NC_):
        s = np.asarray(results[c]["out"], np.float32)  # [B, 128, D]
        for b_ in range(B):
            out[b_, 128 * c:128 * (c + 1), :] = s[b_]
    return out


def kernel(**inputs):
    from concourse.bass_utils import run_bass_kernel_spmd
    nc = build_nc()
    in_maps = make_in_maps(inputs)
    trace = bool(int(os.environ.get("MAMBA_TRACE", "0")))
    res = run_bass_kernel_spmd(nc, in_maps, list(range(NC_)), trace=trace)
    if trace and res.exec_time_ns is not None:
        print(f"HW exec time: {res.exec_time_ns} ns")
    return assemble_output(res.results)


# revision 17
# speedup vs baseline: 1.1523x; 1.1523x over previous
"""Mamba block Trainium2 kernel, 8-way tensor-parallel over d_inner.

Shapes (hardcoded from the problem spec):
  hidden_states [2, 1024, 1024], d_model=1024, d_inner=2048, d_state=16,
  dt_rank=64, d_conv=4.  Each core owns DL=256 d_inner channels (two
  128-partition halves, "dt0"/"dt1", processed merged along the free dim).

Engine assignment (per-core):
  PE   : in_proj / x_proj / dt_proj / out_proj matmuls + y-accumulate over n
         (identity-stationary matmuls into PSUM)
  ACT  : conv taps (scaled copies), Silu, softplus (Exp+Ln), per-n
         exp(A*delta), h fp32->fp16 casts, PSUM->SBUF staging copies
  DVE  : conv tap adds, du/dbu/ch fp16 muls, the sequential scans, gating
  POOL : collectives + chain-breaker memsets only (DVE+POOL share SBUF
         ports; concurrent streaming on both just splits the bandwidth)

All elementwise traffic is uniform fp16 (measured: [128,1024] TT mul 683ns
vs ~2300-3400ns for mixed-dtype variants; scan fp16->fp32 2258ns vs 3942ns
with a bf16 operand).  The scan keeps an fp32 internal state/output.
The scan hot loop runs on [128, 2048] tiles covering both channel halves;
a zeroed dA column at position L breaks the recurrence between them.

Pipelining: head(b0) -> AR1(b0) overlapped with head(b1); delta(b1) emitted
inside scan(b0); gate/out_proj/ReduceScatter of b0 emitted inside scan(b1).
"""
import sys, os
sys.path.insert(0, "/opt/trn_rl_repo")
import numpy as np

import concourse.bass as bass
import concourse.bacc as bacc
import concourse.mybir as mybir
import concourse.tile as tile

F32 = mybir.dt.float32
F16 = mybir.dt.float16
AF = mybir.ActivationFunctionType
OP = mybir.AluOpType

B, L, D, DI, NST, RNK, KC = 2, 1024, 1024, 2048, 16, 64, 4
NC_ = 8
DL = DI // NC_          # 256 local channels
T = B * L               # 2048 tokens
PADL = L + KC - 1       # 1027
L2 = 2 * L              # merged dt0|dt1 free extent


def build_nc():
    nc = bacc.Bacc()
    dp = nc.declare_dram_parameter
    hsT = dp("hsT", [D, T], F16, isOutput=False)             # hidden^T fp16
    wxz = dp("wxzT", [8, 128, 512], F16, isOutput=False)     # in_proj^T k-tiles
    xpw = dp("xpwT", [2, 128, 96], F16, isOutput=False)      # x_proj^T k-tiles
    dtw = dp("dtwT", [RNK, DL], F16, isOutput=False)         # dt_proj^T
    wo = dp("woT", [2, 128, D], F16, isOutput=False)         # out_proj^T k-tiles
    cw = dp("convw", [2, 128, KC], F32, isOutput=False)
    cb = dp("convb", [2, 128, 1], F32, isOutput=False)
    db2 = dp("dtb2p", [2, 128, 1], F32, isOutput=False)      # +2*dt_proj_b
    av = dp("Aneg", [2, 128, NST], F32, isOutput=False)      # -exp(A_log)
    dv = dp("Dvec", [2, 128, 1], F32, isOutput=False)
    idm = dp("ident", [128, 128], F32, isOutput=False)
    outp = dp("out", [B, 128, D], F16, isOutput=True)        # per-core RS slice

    ar1_in = [nc.dram_tensor(f"ar1_in{b}", [96, L], F16) for b in range(B)]
    ar1_out = [nc.dram_tensor(f"ar1_out{b}", [96, L], F16, addr_space="Shared")
               for b in range(B)]
    ar2_in = [nc.dram_tensor(f"ar2_in{b}", [L, D], F16) for b in range(B)]
    ar2_out = [nc.dram_tensor(f"ar2_out{b}", [128, D], F16) for b in range(B)]

    with tile.TileContext(nc) as tc:
        with tc.tile_pool(name="wp", bufs=1) as wp, \
             tc.tile_pool(name="data", bufs=1) as dpool, \
             tc.tile_pool(name="stream", bufs=4) as stream, \
             tc.tile_pool(name="scan", bufs=2) as scp, \
             tc.tile_pool(name="stage", bufs=2) as stg, \
             tc.tile_pool(name="psA", bufs=4, space="PSUM") as psA, \
             tc.tile_pool(name="psy", bufs=2, space="PSUM") as psy:

            # ---- weights / constants ----
            # wxz first (in_proj needs it immediately); the rest are deferred
            # until after the first in_proj k-sweep is emitted.
            wxz_sb = wp.tile([128, 8 * 512], F16)
            for k in range(8):
                nc.sync.dma_start(wxz_sb[:, 512 * k:512 * (k + 1)], wxz[k])
            xpw_sb = wp.tile([128, 2 * 96], F16)
            dtw_sb = wp.tile([RNK, DL], F16)
            wo_sb = wp.tile([128, 2 * D], F16)
            cw_sb = wp.tile([128, 2 * KC], F32)
            cb_sb = wp.tile([128, 2], F32)
            db2_sb = wp.tile([128, 2], F32)
            av_sb = wp.tile([128, 2 * NST], F32)
            dv_sb = wp.tile([128, 2], F32)
            id_sb = wp.tile([128, 128], F32)
            id16 = wp.tile([128, 128], F16)

            def emit_weight_tail():
                nc.sync.dma_start(dtw_sb[:], dtw[:])
                for k in range(2):
                    nc.sync.dma_start(xpw_sb[:, 96 * k:96 * (k + 1)], xpw[k])
                    nc.sync.dma_start(wo_sb[:, D * k:D * (k + 1)], wo[k])
                    nc.sync.dma_start(cw_sb[:, KC * k:KC * (k + 1)], cw[k])
                    nc.sync.dma_start(cb_sb[:, k:k + 1], cb[k])
                    nc.sync.dma_start(db2_sb[:, k:k + 1], db2[k])
                    nc.sync.dma_start(av_sb[:, NST * k:NST * (k + 1)], av[k])
                    nc.sync.dma_start(dv_sb[:, k:k + 1], dv[k])
                nc.sync.dma_start(id_sb[:], idm[:])
                nc.vector.tensor_copy(id16[:], id_sb[:])

            cwv = cw_sb.rearrange("p (k m) -> p k m", k=2)
            wxzv = wxz_sb.rearrange("p (k m) -> p k m", k=8)
            wov = wo_sb.rearrange("p (k m) -> p k m", k=2)

            # ---- persistent per-batch activations (dt-merged free dim) ----
            xpad = [[dpool.tile([128, PADL], F16, name=f"xpad{b}_{i}")
                     for i in range(2)] for b in range(B)]
            sg16 = [dpool.tile([128, L2], F16, name=f"sg{b}") for b in range(B)]
            u16 = [dpool.tile([128, L2], F16, name=f"u{b}") for b in range(B)]
            delta = [dpool.tile([128, L2], F32, name=f"delta{b}")
                     for b in range(B)]
            du16 = [dpool.tile([128, L2], F16, name=f"du{b}") for b in range(B)]
            yg16 = [dpool.tile([128, L2], F16, name=f"yg{b}") for b in range(B)]
            xdbl = [dpool.tile([RNK, L], F16, name=f"xdbl{b}") for b in range(B)]

            def emit_head(b):
                for dt_ in range(2):
                    nc.vector.memset(xpad[b][dt_][:, 0:KC - 1], 0.0)
                # in_proj in half-token sweeps; conv/silu/x_proj follow each
                # sweep so x_proj(b) is ready as soon as in_proj(b) drains.
                for th in range(2):
                    hts = slice(L * b + 512 * th, L * b + 512 * (th + 1))
                    ps4 = [psA.tile([128, 512], F32, name=f"psip{i}", tag="psA")
                           for i in range(4)]   # x0 x1 z0 z1
                    for k in range(8):
                        hk = stream.tile([128, 512], F16, name="hk", tag="hst")
                        nc.sync.dma_start(hk[:], hsT[128 * k:128 * (k + 1), hts])
                        for g in range(4):
                            nc.tensor.matmul(
                                ps4[g][:], wxzv[:, k, 128 * g:128 * (g + 1)],
                                hk[:], start=(k == 0), stop=(k == 7))
                    if b == 0 and th == 0:
                        emit_weight_tail()
                    o = KC - 1 + 512 * th
                    nc.scalar.copy(xpad[b][0][:, o:o + 512], ps4[0][:])
                    nc.scalar.copy(xpad[b][1][:, o:o + 512], ps4[1][:])
                    nc.scalar.activation(sg16[b][:, 512 * th:512 * (th + 1)],
                                         ps4[2][:], AF.Silu)
                    nc.scalar.activation(
                        sg16[b][:, L + 512 * th:L + 512 * (th + 1)],
                        ps4[3][:], AF.Silu)
                    # conv + silu for this half-token range
                    c0 = 512 * th
                    for dt_ in range(2):
                        taps = [stg.tile([128, 512], F16, name=f"tap{dt_}_{k}",
                                         tag=f"tap{dt_}_{k}", bufs=2)
                                for k in range(KC)]
                        for k in range(KC):
                            nc.scalar.activation(
                                taps[k][:], xpad[b][dt_][:, c0 + k:c0 + k + 512],
                                AF.Identity, scale=cwv[:, dt_, k:k + 1])
                        t01 = stg.tile([128, 512], F16, name="t01",
                                       tag=f"t01_{dt_}", bufs=2)
                        t23 = stg.tile([128, 512], F16, name="t23",
                                       tag=f"t23_{dt_}", bufs=2)
                        nc.vector.tensor_add(t01[:], taps[0][:], taps[1][:])
                        nc.vector.tensor_add(t23[:], taps[2][:], taps[3][:])
                        acc = stg.tile([128, 512], F16, name="acc",
                                       tag=f"acc{dt_}", bufs=2)
                        nc.vector.tensor_add(acc[:], t01[:], t23[:])
                        nc.scalar.activation(
                            u16[b][:, L * dt_ + c0:L * dt_ + c0 + 512],
                            acc[:], AF.Silu, bias=cb_sb[:, dt_:dt_ + 1])
                # x_proj after both sweeps so its matmuls never block the
                # PE queue while waiting on the conv chain
                for th in range(2):
                    c0 = 512 * th
                    ps96 = psA.tile([96, 512], F32, name="ps96", tag="psA")
                    for k in range(2):
                        nc.tensor.matmul(
                            ps96[:], xpw_sb[:, 96 * k:96 * (k + 1)],
                            u16[b][:, L * k + c0:L * k + c0 + 512],
                            start=(k == 0), stop=(k == 1))
                    st = stg.tile([96, 512], F16, name="st_xp", tag="xp")
                    nc.scalar.copy(st[:], ps96[:])
                    nc.sync.dma_start(
                        ar1_in[b][:, 512 * th:512 * (th + 1)], st[:])
                if os.environ.get("MAMBA_NO_AR"):
                    nc.sync.dma_start(ar1_out[b][:], ar1_in[b][:])
                else:
                    nc.gpsimd.collective_compute(
                        "AllReduce", OP.add,
                        replica_groups=[list(range(NC_))],
                        ins=[ar1_in[b][:]], outs=[ar1_out[b][:]])

            def emit_delta(b):
                nc.sync.dma_start(xdbl[b][:], ar1_out[b][0:RNK, :])
                for dt_ in range(2):
                    dsl = delta[b][:, L * dt_:L * (dt_ + 1)]
                    psd = psA.tile([128, 512], F32, name="psda", tag="psA")
                    psdb = psA.tile([128, 512], F32, name="psdb", tag="psA")
                    nc.tensor.matmul(psd[:], dtw_sb[:, 128 * dt_:128 * (dt_ + 1)],
                                     xdbl[b][:, 0:512], start=True, stop=True)
                    nc.tensor.matmul(psdb[:], dtw_sb[:, 128 * dt_:128 * (dt_ + 1)],
                                     xdbl[b][:, 512:L], start=True, stop=True)
                    # delta = softplus(dt_raw + 2*dt_proj_b) = ln(exp(..)+1)
                    nc.scalar.activation(dsl[:, 0:512], psd[:], AF.Exp,
                                         bias=db2_sb[:, dt_:dt_ + 1])
                    nc.scalar.activation(dsl[:, 512:L], psdb[:], AF.Exp,
                                         bias=db2_sb[:, dt_:dt_ + 1])
                # Ln ops grouped after both Exp blocks (one table switch)
                nc.scalar.activation(delta[b][:], delta[b][:], AF.Ln, bias=1.0)
                # du = delta * u   (fp32 x fp16 -> fp16, merged)
                nc.vector.tensor_mul(du16[b][:], delta[b][:], u16[b][:])

            def emit_gate(b):
                # y = (u*D + y_scan) * silu(z)  -- frees py[b] PSUM banks
                for dt_ in range(2):
                    lsl = slice(L * dt_, L * (dt_ + 1))
                    t16 = stg.tile([128, L], F16, name="t16", tag="t16")
                    nc.vector.scalar_tensor_tensor(
                        t16[:], u16[b][:, lsl], dv_sb[:, dt_:dt_ + 1],
                        py[b][dt_][:], op0=OP.mult, op1=OP.add)
                    nc.vector.tensor_mul(yg16[b][:, lsl], t16[:],
                                         sg16[b][:, lsl])

            def emit_out_tile(b, tt):
                pso = [psA.tile([128, 512], F32, name=f"pso{i}", tag="psA")
                       for i in range(2)]
                for k in range(2):
                    ysl = yg16[b][:, L * k + 128 * tt:L * k + 128 * (tt + 1)]
                    for hh in range(2):
                        nc.tensor.matmul(
                            pso[hh][:], ysl,
                            wov[:, k, 512 * hh:512 * (hh + 1)],
                            start=(k == 0), stop=(k == 1))
                st = stg.tile([128, D], F16, name="st_op", tag="op")
                nc.scalar.copy(st[:, 0:512], pso[0][:])
                nc.scalar.copy(st[:, 512:D], pso[1][:])
                nc.sync.dma_start(ar2_in[b][128 * tt:128 * (tt + 1), :], st[:])

            def emit_rs(b):
                if os.environ.get("MAMBA_NO_AR"):
                    nc.sync.dma_start(ar2_out[b][:], ar2_in[b][0:128, :])
                else:
                    nc.gpsimd.collective_compute(
                        "ReduceScatter", OP.add,
                        replica_groups=[list(range(NC_))],
                        ins=[ar2_in[b][:]], outs=[ar2_out[b][:]])
                nc.sync.dma_start(outp[b], ar2_out[b][:])

            py = [None, None]
            # per-n scan-state carries for the b1 half-token passes
            carry = [wp.tile([128, NST], F32, name="carry0"),
                     wp.tile([128, NST], F32, name="carry1")]

            def emit_scan_pass(b, pass_, width, insert=None):
                """One n-loop over token columns [c0, c0+width) of batch b,
                dt-merged: op tiles are [128, 2*width] = dt0|dt1.
                pass_ 0 with width=L is the full-batch single pass.
                For width=512: pass 0 saves end-of-pass state into carry[];
                pass 1 seeds from it (initial= for dt0, dbu fixup for dt1)."""
                c0 = 512 * pass_
                W2 = 2 * width
                for n in range(NST):
                    bct = scp.tile([128, W2], F16, name="bct", tag="bct",
                                   bufs=3)
                    bv = bct.rearrange("p (two l) -> p two l", two=2)
                    nc.sync.dma_start(
                        bv[:, 0], ar1_out[b][RNK + n:RNK + n + 1, c0:c0 + width]
                        .broadcast_to((128, width)))
                    nc.sync.dma_start(
                        bv[:, 1],
                        ar1_out[b][RNK + NST + n:RNK + NST + n + 1, c0:c0 + width]
                        .broadcast_to((128, width)))
                    brep = bct[:, 0:width].unsqueeze(1).broadcast_to(
                        (128, 2, width))
                    crep = bct[:, width:W2].unsqueeze(1).broadcast_to(
                        (128, 2, width))
                    dA16 = scp.tile([128, W2], F16, name="dA", tag="dA", bufs=3)
                    nc.scalar.activation(
                        dA16[:, 0:width], delta[b][:, c0:c0 + width], AF.Exp,
                        scale=av_sb[:, n:n + 1])
                    # dt1 exp: skip col `width` only when its dA must be zero
                    # with no carry (start of the dt1 sequence)
                    d1lo = width + (0 if pass_ == 1 else 1)
                    nc.scalar.activation(
                        dA16[:, d1lo:W2],
                        delta[b][:, L + c0 + (d1lo - width):L + c0 + width],
                        AF.Exp, scale=av_sb[:, NST + n:NST + n + 1])
                    dbu = scp.tile([128, W2], F16, name="dbu", tag="dbu",
                                   bufs=3)
                    du_v = du16[b].rearrange("p (two l) -> p two l", two=2)
                    dbu_v = dbu.rearrange("p (two l) -> p two l", two=2)
                    nc.vector.tensor_mul(dbu_v[:], du_v[:, :, c0:c0 + width],
                                         brep)
                    if pass_ == 1:
                        # seed dt1 from carried state, then break the chain
                        nc.vector.scalar_tensor_tensor(
                            dbu[:, width:width + 1], dA16[:, width:width + 1],
                            carry[1][:, n:n + 1], dbu[:, width:width + 1],
                            op0=OP.mult, op1=OP.add)
                    nc.vector.memset(dA16[:, width:width + 1], 0.0)
                    h = scp.tile([128, W2], F32, name="h", tag="h", bufs=2)
                    init = carry[0][:, n:n + 1] if pass_ == 1 else 0.0
                    nc.vector.tensor_tensor_scan(
                        h[:], dA16[:], dbu[:], init, op0=OP.mult, op1=OP.add)
                    if width != L and pass_ == 0:
                        nc.scalar.copy(carry[0][:, n:n + 1],
                                       h[:, width - 1:width])
                        nc.scalar.copy(carry[1][:, n:n + 1],
                                       h[:, W2 - 1:W2])
                    h16 = scp.tile([128, W2], F16, name="h16", tag="h16",
                                   bufs=2)
                    nc.scalar.copy(h16[:], h[:])
                    ch = scp.tile([128, W2], F16, name="ch", tag="ch", bufs=2)
                    h16_v = h16.rearrange("p (two l) -> p two l", two=2)
                    ch_v = ch.rearrange("p (two l) -> p two l", two=2)
                    nc.vector.tensor_mul(ch_v[:], h16_v[:], crep)
                    for dt_ in range(2):
                        for q0 in range(0, width, 512):
                            nc.tensor.matmul(
                                py[b][dt_][:, c0 + q0:c0 + q0 + 512], id16[:],
                                ch[:, width * dt_ + q0:width * dt_ + q0 + 512],
                                start=(n == 0), stop=(n == NST - 1))
                    if insert:
                        insert(n)

            def emit_gate_half(b, pass_, width):
                c0 = 512 * pass_
                for dt_ in range(2):
                    t16 = stg.tile([128, width], F16, name="t16", tag="t16")
                    nc.vector.scalar_tensor_tensor(
                        t16[:], u16[b][:, L * dt_ + c0:L * dt_ + c0 + width],
                        dv_sb[:, dt_:dt_ + 1], py[b][dt_][:, c0:c0 + width],
                        op0=OP.mult, op1=OP.add)
                    nc.vector.tensor_mul(
                        yg16[b][:, L * dt_ + c0:L * dt_ + c0 + width],
                        t16[:], sg16[b][:, L * dt_ + c0:L * dt_ + c0 + width])

            emit_head(0)
            emit_head(1)
            emit_delta(0)
            py[0] = [psy.tile([128, L], F32, name=f"py0_{i}", tag="psy")
                     for i in range(2)]

            def ins_b0(n):
                if n == 2:
                    emit_delta(1)
            emit_scan_pass(0, 0, L, insert=ins_b0)
            emit_gate_half(0, 0, L)
            py[1] = [psy.tile([128, L], F32, name=f"py1_{i}", tag="psy")
                     for i in range(2)]

            def ins_b1(n):
                # b0's out_proj + RS, hidden under scan(b1)
                if 2 <= n <= 9:
                    emit_out_tile(0, n - 2)
                if n == 11:
                    emit_rs(0)
            emit_scan_pass(1, 0, L, insert=ins_b1)
            emit_gate_half(1, 0, L)
            for tt in range(8):
                emit_out_tile(1, tt)
            emit_rs(1)
    nc.finalize()
    return nc


def make_in_maps(inputs):
    hs = np.asarray(inputs["hidden_states"], np.float32)
    ipw = np.asarray(inputs["in_proj_w"], np.float32)
    cw = np.asarray(inputs["conv_w"], np.float32)
    cb = np.asarray(inputs["conv_b"], np.float32)
    xpw = np.asarray(inputs["x_proj_w"], np.float32)
    dtw = np.asarray(inputs["dt_proj_w"], np.float32)
    dtb = np.asarray(inputs["dt_proj_b"], np.float32)
    alog = np.asarray(inputs["A_log"], np.float32)
    dvec = np.asarray(inputs["D"], np.float32)
    wo = np.asarray(inputs["out_proj_w"], np.float32)

    hsT = np.ascontiguousarray(hs.transpose(2, 0, 1).reshape(D, T)).astype(np.float16)
    ident = np.eye(128, dtype=np.float32)

    in_maps = []
    for c in range(NC_):
        sl = slice(DL * c, DL * (c + 1))
        wxzT = np.concatenate([ipw[sl].T, ipw[DI + DL * c: DI + DL * (c + 1)].T],
                              axis=1)                      # [1024, 512]
        m = {
            "hsT": hsT,
            "wxzT": np.ascontiguousarray(wxzT.reshape(8, 128, 512)).astype(np.float16),
            "xpwT": np.ascontiguousarray(xpw[:, sl].T.reshape(2, 128, 96)).astype(np.float16),
            "dtwT": np.ascontiguousarray(dtw[sl].T).astype(np.float16),
            "woT": np.ascontiguousarray(wo[:, sl].T.reshape(2, 128, D)).astype(np.float16),
            "convw": np.ascontiguousarray(cw[sl, 0, :].reshape(2, 128, KC)),
            "convb": np.ascontiguousarray(cb[sl].reshape(2, 128, 1)),
            "dtb2p": np.ascontiguousarray((2.0 * dtb[sl]).reshape(2, 128, 1)),
            "Aneg": np.ascontiguousarray((-np.exp(alog[sl])).reshape(2, 128, NST)),
            "Dvec": np.ascontiguousarray(dvec[sl].reshape(2, 128, 1)),
            "ident": ident,
        }
        in_maps.append(m)
    return in_maps


def assemble_output(results):
    out = np.zeros((B, L, D), np.float32)
    for c in range(NC_):
        s = np.asarray(results[c]["out"], np.float32)  # [B, 128, D]
        for b_ in range(B):
            out[b_, 128 * c:128 * (c + 1), :] = s[b_]
    return out


def kernel(**inputs):
    from concourse.bass_utils import run_bass_kernel_spmd
    nc = build_nc()
    in_maps = make_in_maps(inputs)
    trace = bool(int(os.environ.get("MAMBA_TRACE", "0")))
    res = run_bass_kernel_spmd(nc, in_maps, list(range(NC_)), trace=trace)
    if trace and res.exec_time_ns is not None:
        print(f"HW exec time: {res.exec_time_ns} ns")
    return assemble_output(res.results)
